# revision 74
# baseline (speedup 1.0000x reference)
"""Trainium2 Bass kernel for nn_MultiHeadAttention (B=8, S=1024, D=1024, H=16).

Sharding: data-parallel over batch — 8 NeuronCores, one batch element each;
weights replicated. No collectives needed.

Per-core plan (all matmul contractions on the partition dim, float32r PE
datapath = full rate at N=512, ~1e-4 rel err):

  phase A: PE-transpose x3/x1/x2 128x128 blocks (identity matmul, f32r
           datapath = 1.5 cyc/row) into x^T layouts; project
           v = (x3T as lhsT) @ Wk (natural [S, D]), interleaved with the
           x1/x2 transposes.  Q/K biases are per-partition in the out^T
           layout so they ride the PSUM evacuation as tensor_scalar_add;
           the V bias (== bk, reference bug) commutes through Wo and is
           folded on the host into cb = bk @ Wo + bo (exact algebra), so
           no bias matmuls exist anywhere.  The key/pad mask is folded
           into an augmented value matrix vaug = [m * v | m] so masking
           AND the softmax denominator ride the PV matmul for free
           (P*m @ v == P @ (m*v), denom = P @ m).
  phase B: per head-pair p: q^T/k^T projections for pair p+1 are emitted
           as generators interleaved into pair p's attention loop (PE
           slack absorbs them, ACT stays saturated).  Scores
           S^T[k,q] = kT-slice^T @ qT-slice (K=64, two heads row-tiled
           into PE row-groups); P^T = exp(S^T/8) via one [128,1024] ACT
           op per k-tile straight out of PSUM (no max-subtraction:
           |scores/8| <= ~7 is fp32-safe); O^T_aug[65, q] +=
           vaug-slice^T @ P^T accumulated over k-tiles (row 64 = softmax
           denominator); 1/denom = exp(-ln(d)) on ACT (DVE reciprocal is
           8 cyc/elem, ACT Rsqrt/Reciprocal banned); broadcast along
           partitions via DRAM-roundtrip gpsimd DMA; normalize into oT.
           Exp and Ln are pinned to one activation-table set (33 table
           reloads avoided).
  phase 3: out = (oT as lhsT) @ Wo, then cb = bk@Wo + bo rides a K=1
           bias matmul and the residual x1 rides an identity matmul, so
           the whole pre-LN sum materializes in PSUM with zero DVE ops;
           LayerNorm via bn_stats/bn_aggr straight out of PSUM + Sqrt +
           DVE reciprocal, then two fused scalar_tensor_tensor ops
           ((x-mu)*gamma, *rstd+beta) with gamma/beta from
           DMA-partition-broadcast tiles.

Cost-model timeline (TimelineSim): ~315.5 us/core (PE ~78% busy at 248
us; phase B is ACT-bound on the exp floor ~201 us; the rest is startup
DMA and the phase-3 LN tail).  The Wo loads ride the Pool (SWDGE) queue
so they overlap the last attention pair instead of queueing on SP
behind the dependency-stalled reciprocal round-trip DMAs (the same
reroute hurts for rdr/xres/y — Pool descriptor-gen is slower and those
sit on latency chains).  Measured correct at rel err 1.9e-04 vs the
fp32 reference (the f32r transposes add ~1.5e-04 over the previous
all-f32 path's 4.9e-05).  Phase A runs all transposes and the
v-projection through one shared 8-bank PSUM pool (no pool-churn
barriers); weight tiles prefetch 8-10 deep to hide HWDGE latency.  All
projection loops are di-outer so each weight tile is DMA'd once and
feeds both 512-wide chunks back-to-back (halves W traffic and reuses
the PE stationary).
"""
import sys

if "/opt/trn_rl_repo" not in sys.path:
    sys.path.insert(0, "/opt/trn_rl_repo")

import numpy as np

B, S, D, H = 8, 1024, 1024, 16
DK = D // H          # 64
NP = H // 2          # 8 head pairs
ST = S // 128        # 8 s-tiles (also k-tiles)
DT = D // 128        # 8 d-tiles
NC = S // 512        # 2 chunks of 512
VW = DK + 1          # 65: augmented head width
EPS = 1e-5

_BUILT = None


def _build():
    import concourse.bass as bass  # noqa: F401
    import concourse.tile as tile
    from concourse import bacc, mybir
    from concourse.masks import make_identity

    # Keep Exp and Ln in one activation-table set: remove them from every
    # other set (set order/indices preserved) so the table-load pass resolves
    # both to natural_log_exp_and_others instead of thrashing 33 reloads.
    AFt = mybir.ActivationFunctionType
    if not getattr(bacc, "_mha_act_tables_patched", False):
        orig_gat = bacc.get_activation_tables

        def _patched_gat(arch):
            t = dict(orig_gat(arch))
            for name, fns in t.items():
                if name != "natural_log_exp_and_others":
                    t[name] = {f for f in fns if f not in (AFt.Exp, AFt.Ln)}
            return t

        bacc.get_activation_tables = _patched_gat
        bacc._mha_act_tables_patched = True

    f32 = mybir.dt.float32
    f32r = mybir.dt.float32r
    AF = mybir.ActivationFunctionType

    nc = bacc.Bacc("TRN2", target_bir_lowering=False, debug=False, num_devices=B)

    # x inputs as float32r: same bytes as f32, PE transposes run 1.5 cyc/row
    # instead of 2.0
    x1_ap = nc.dram_tensor("x1", [S, D], f32r, kind="ExternalInput").ap()
    x2_ap = nc.dram_tensor("x2", [S, D], f32r, kind="ExternalInput").ap()
    x3_ap = nc.dram_tensor("x3", [S, D], f32r, kind="ExternalInput").ap()
    wq_ap = nc.dram_tensor("wq", [D, D], f32r, kind="ExternalInput").ap()
    wk_ap = nc.dram_tensor("wk", [D, D], f32r, kind="ExternalInput").ap()
    wo_ap = nc.dram_tensor("wo", [D, D], f32r, kind="ExternalInput").ap()
    bq_ap = nc.dram_tensor("bq", [1, D], f32, kind="ExternalInput").ap()
    bk_ap = nc.dram_tensor("bk", [1, D], f32, kind="ExternalInput").ap()
    # cb = bk @ Wo + bo, precomputed on host: the V-bias (reference reuses
    # bk for V) passes through Wo as a constant row, so it folds with bo
    cb_ap = nc.dram_tensor("cb", [1, D], f32r, kind="ExternalInput").ap()
    gamma_ap = nc.dram_tensor("gamma", [1, D], f32, kind="ExternalInput").ap()
    beta_ap = nc.dram_tensor("beta", [1, D], f32, kind="ExternalInput").ap()
    mf_ap = nc.dram_tensor("mf", [S, 1], f32, kind="ExternalInput").ap()
    y_ap = nc.dram_tensor("y", [S, D], f32, kind="ExternalOutput").ap()

    with tile.TileContext(nc) as tc:
        with tc.tile_pool(name="persist", bufs=1) as persist:
            smalls = persist.tile([128, 512], f32)
            ones_p = smalls[:, 128:144]      # [128, 16] of ones
            eps_t = smalls[:, 144:145]
            m_sb = smalls[:, 145:153]        # [128, ST] mask per k-tile
            nc.vector.memset(ones_p, 1.0)
            nc.vector.memset(eps_t, EPS)
            # f32r identity: transposes of f32r inputs run 1.5 cyc/row.
            # gpsimd memset can't target f32r, so build in f32 and copy.
            ident_f = persist.tile([128, 128], f32)
            make_identity(nc, ident_f[:])
            ident_t = persist.tile([128, 128], f32r)
            ident = ident_t[:]
            nc.vector.tensor_copy(ident, ident_f[:])
            ones_f = persist.tile([1, 512], f32)
            nc.vector.memset(ones_f[:], 1.0)
            ones_r = persist.tile([1, 512], f32r)
            nc.vector.tensor_copy(ones_r[:], ones_f[:])
            # per-pair bias columns: bqT[part, p] = bq[p*128 + part]
            bqT_sb = persist.tile([128, NP], f32)
            bkT_sb = persist.tile([128, NP], f32)

            # oT outlives the phase-1/2 tensors: allocate below them
            oT_pool = tc.alloc_tile_pool(name="oTp", bufs=1)
            oT = oT_pool.tile([128, DT * S], f32r)
            # live through phases 1-2, released before phase 3
            qkv_pool = tc.alloc_tile_pool(name="qkv", bufs=1)
            x1T = qkv_pool.tile([128, DT * S], f32r)
            x2T = qkv_pool.tile([128, DT * S], f32r)
            vaug = qkv_pool.tile([128, ST * H * VW], f32r)  # k-tile t at t*H*VW

            # ------- phase A: transposes; v-projection (mask-augmented) -------
            with tc.tile_pool(name="pA_x3", bufs=1) as x3_pool, \
                 tc.tile_pool(name="pA_w", bufs=6) as w_pool, \
                 tc.tile_pool(name="pA_stage", bufs=10) as stage:

                pA_ps = tc.alloc_tile_pool(name="pA_ps", bufs=8, space="PSUM")

                def transpose_in(x_ap, xT):
                    # xT layout [128, DT*S]: d-tile dt at cols [dt*S + s]
                    xT3 = xT[:].rearrange("p (d s) -> p d s", s=S)
                    for st in range(ST):
                        for half in range(2):
                            xs = stage.tile([128, 512], f32r, name="xs", tag="xs")
                            nc.sync.dma_start(
                                xs[:], x_ap[st * 128:(st + 1) * 128,
                                            half * 512:(half + 1) * 512])
                            tp = pA_ps.tile([128, 512], f32r, name="tp", tag="ps512")
                            for j in range(4):
                                nc.tensor.transpose(
                                    tp[:, j * 128:(j + 1) * 128],
                                    xs[:, j * 128:(j + 1) * 128], ident)
                            dst = xT3[:, half * 4:half * 4 + 4,
                                      st * 128:(st + 1) * 128]
                            nc.scalar.copy(dst, tp[:].rearrange(
                                "p (b c) -> p b c", b=4))

                def v_proj_half(x3T, c):
                    # v natural [S, D] + augmentation with the mask; no V
                    # bias here — it folds into cb = bk @ Wo + bo (host)
                    if True:
                        pss = [pA_ps.tile([128, 512], f32, name=f"vp{i}",
                                          tag="ps512") for i in range(ST)]
                        for di in range(DT):
                            wd = w_pool.tile([128, 512], f32r,
                                             name="wdv", tag="wd")
                            nc.sync.dma_start(
                                wd[:], wk_ap[di * 128:(di + 1) * 128,
                                             c * 512:(c + 1) * 512])
                            for st in range(ST):
                                nc.tensor.matmul(
                                    pss[st][:],
                                    x3T[:, di * S + st * 128:
                                        di * S + (st + 1) * 128],
                                    wd[:], start=(di == 0),
                                    stop=(di == DT - 1))
                        for st in range(ST):
                            va = vaug[:, st * H * VW:(st + 1) * H * VW].rearrange(
                                "p (h e) -> p h e", e=VW)
                            nc.vector.tensor_scalar_mul(
                                va[:, 8 * c:8 * (c + 1), 0:DK],
                                pss[st][:].rearrange("p (h e) -> p h e", e=DK),
                                m_sb[:, st:st + 1])
                            if c == 0:
                                nc.vector.tensor_scalar_mul(
                                    va[:, :, DK:VW],
                                    ones_p.rearrange("p (h e) -> p h e", e=1),
                                    m_sb[:, st:st + 1])

                x3T = x3_pool.tile([128, DT * S], f32r)
                transpose_in(x3_ap, x3T)
                nc.sync.dma_start(m_sb, mf_ap.rearrange(
                    "(t p) o -> p (t o)", p=128))
                nc.sync.dma_start(bqT_sb[:], bq_ap.rearrange(
                    "o (n p) -> p (n o)", p=128))
                nc.sync.dma_start(bkT_sb[:], bk_ap.rearrange(
                    "o (n p) -> p (n o)", p=128))
                v_proj_half(x3T, 0)
                transpose_in(x1_ap, x1T)
                v_proj_half(x3T, 1)
                transpose_in(x2_ap, x2T)
                pA_ps.release()

            # --- phase B: per-pair q/k projection pipelined with attention ---
            with tc.tile_pool(name="pB_qk", bufs=2) as qk_pool, \
                 tc.tile_pool(name="pB_w", bufs=8) as w2_pool, \
                 tc.tile_pool(name="pB_P", bufs=5) as P_pool, \
                 tc.tile_pool(name="pB_scr", bufs=3) as scr_pool, \
                 tc.tile_pool(name="pB_dr", bufs=8, space="DRAM") as dr_pool, \
                 tc.tile_pool(name="pB_pps", bufs=2, space="PSUM") as proj_ps, \
                 tc.tile_pool(name="pB_sps", bufs=2, space="PSUM") as s_ps, \
                 tc.tile_pool(name="pB_ops", bufs=3, space="PSUM") as o_ps:

                def proj_pair_gen(p, w_ap_, bT_sb, xT, out):
                    # out[r, s] = sum_di (W[di, p-block] as lhsT) @ xT[di] + b
                    # generator: yields after each di so the caller can
                    # interleave these into the attention PE stream.
                    # di-outer: each weight tile is DMA'd once and feeds both
                    # q-chunks back-to-back (stationary reuse on the PE).
                    # The bias is per-partition in the out^T layout, so it
                    # rides the PSUM->SBUF evacuation as a tensor_scalar_add
                    # (no bias matmul).
                    pps = [proj_ps.tile([128, 512], f32, name=f"pp{c}",
                                        tag="pp") for c in range(NC)]
                    for di in range(DT):
                        wd = w2_pool.tile([128, 128], f32r,
                                          name="wd2", tag="wd2")
                        nc.sync.dma_start(
                            wd[:], w_ap_[di * 128:(di + 1) * 128,
                                         p * 128:(p + 1) * 128])
                        for c in range(NC):
                            nc.tensor.matmul(
                                pps[c][:], wd[:],
                                xT[:, di * S + c * 512:di * S + (c + 1) * 512],
                                start=(di == 0), stop=(di == DT - 1))
                        yield
                    for c in range(NC):
                        nc.vector.tensor_scalar_add(
                            out[:, c * 512:(c + 1) * 512], pps[c][:],
                            bT_sb[:, p:p + 1])
                        yield

                def proj_pair(p):
                    q_t = qk_pool.tile([128, S], f32r, name=f"q{p}", tag="q")
                    k_t = qk_pool.tile([128, S], f32r, name=f"k{p}", tag="k")
                    gq = proj_pair_gen(p, wq_ap, bqT_sb, x1T, q_t)
                    gk = proj_pair_gen(p, wk_ap, bkT_sb, x2T, k_t)
                    return q_t, k_t, gq, gk

                def drain_gen(g, n=1000):
                    for _ in range(n):
                        try:
                            next(g)
                        except StopIteration:
                            return

                qTp, kTp, gq, gk = proj_pair(0)
                drain_gen(gq)
                drain_gen(gk)
                for p in range(NP):
                    # next pair's projections, interleaved into this pair's
                    # attention loop (PE slack absorbs them; ACT stays hot)
                    if p + 1 < NP:
                        qTn, kTn, gq, gk = proj_pair(p + 1)
                    else:
                        qTn = kTn = gq = gk = None
                    for c in range(NC):
                        oaugA = o_ps.tile([VW, 512], f32, name="oaugA", tag="oaug")
                        oaugB = o_ps.tile([VW, 512], f32, name="oaugB", tag="oaug")
                        for kt in range(ST):
                            sc = s_ps.tile([128, 1024], f32, name="sc", tag="sc")
                            nc.tensor.matmul(
                                sc[:, 0:512],
                                kTp[0:64, kt * 128:(kt + 1) * 128],
                                qTp[0:64, c * 512:(c + 1) * 512],
                                start=True, stop=True)
                            nc.tensor.matmul(
                                sc[:, 512:1024],
                                kTp[64:128, kt * 128:(kt + 1) * 128],
                                qTp[64:128, c * 512:(c + 1) * 512],
                                start=True, stop=True)
                            Pt = P_pool.tile([128, 1024], f32r, name="Pt", tag="Pt")
                            nc.scalar.activation(Pt[:], sc[:], AF.Exp,
                                                 scale=1.0 / float(np.sqrt(DK)))
                            base = kt * H * VW
                            nc.tensor.matmul(
                                oaugA[:],
                                vaug[:, base + 2 * p * VW:base + (2 * p + 1) * VW],
                                Pt[:, 0:512],
                                start=(kt == 0), stop=(kt == ST - 1))
                            nc.tensor.matmul(
                                oaugB[:],
                                vaug[:, base + (2 * p + 1) * VW:
                                     base + (2 * p + 2) * VW],
                                Pt[:, 512:1024],
                                start=(kt == 0), stop=(kt == ST - 1))
                            if gq is not None:
                                n = 1 if (c == 0 or kt < 2) else 0
                                drain_gen(gq, n)
                                drain_gen(gk, n)
                        for h_loc, oaug in ((0, oaugA), (1, oaugB)):
                            # free the PSUM bank fast: stash O rows, ln the
                            # denominator; recip + broadcast happen off-bank
                            stash = scr_pool.tile([64, 512], f32,
                                                  name="stash", tag="stash")
                            nc.vector.tensor_copy(stash[:], oaug[0:64, :])
                            rec = scr_pool.tile([1, 512], f32,
                                                name="rec", tag="rec")
                            nc.scalar.activation(rec[:], oaug[64:65, :], AF.Ln)
                            nc.scalar.activation(rec[:], rec[:], AF.Exp,
                                                 scale=-1.0)
                            rdr = dr_pool.tile([1, 512], f32, name="rdr",
                                               tag="rdr")
                            nc.sync.dma_start(rdr[:], rec[:])
                            rbc = scr_pool.tile([64, 512], f32,
                                                name="rbc", tag="rbc")
                            nc.gpsimd.dma_start(rbc[:],
                                                rdr[:].partition_broadcast(64))
                            nc.vector.tensor_mul(
                                oT[h_loc * 64:(h_loc + 1) * 64,
                                   p * S + c * 512:p * S + (c + 1) * 512],
                                stash[:], rbc[:])
                    if gq is not None:
                        drain_gen(gq)
                        drain_gen(gk)
                        qTp, kTp = qTn, kTn
            qkv_pool.release()

            # ---------------- phase 3: out-proj + residual + LayerNorm --------
            with tc.tile_pool(name="p3_w", bufs=1) as w3_pool, \
                 tc.tile_pool(name="p3_stage", bufs=2) as stage3, \
                 tc.tile_pool(name="p3_t", bufs=2) as t_pool, \
                 tc.tile_pool(name="p3_ln", bufs=4) as ln_pool, \
                 tc.tile_pool(name="p3_ps", bufs=3, space="PSUM") as ps3:
                wo_sb = w3_pool.tile([128, DT * D], f32r)
                for dt in range(DT):
                    # Pool (SWDGE) queue: nearly idle in late phase B, so
                    # these start as soon as the freed SBUF range's last
                    # reader retires instead of queueing on SP behind the
                    # dependency-stalled reciprocal round-trip DMAs
                    nc.gpsimd.dma_start(wo_sb[:, dt * D:(dt + 1) * D],
                                        wo_ap[dt * 128:(dt + 1) * 128, :])
                # cb = bk@Wo + bo rides a K=1 bias matmul (PE has slack in
                # this phase; DVE is the phase-3 bottleneck)
                cb_sb = w3_pool.tile([1, D], f32r)
                nc.sync.dma_start(cb_sb[:], cb_ap[:])
                gamma_bc = w3_pool.tile([128, D], f32)
                nc.gpsimd.dma_start(gamma_bc[:], gamma_ap.partition_broadcast(128))
                beta_bc = w3_pool.tile([128, D], f32)
                nc.gpsimd.dma_start(beta_bc[:], beta_ap.partition_broadcast(128))
                for qt in range(ST):
                    # prefetch the residual tile; it joins the PSUM
                    # accumulation as an identity matmul (PE has slack here,
                    # DVE is the phase bottleneck)
                    xres = stage3.tile([128, D], f32r, name="xres", tag="xres")
                    nc.sync.dma_start(xres[:], x1_ap[qt * 128:(qt + 1) * 128, :])
                    ps = ps3.tile([128, 1024], f32, name="ps", tag="ps3")
                    for di in range(DT):
                        for c in range(NC):
                            nc.tensor.matmul(
                                ps[:, c * 512:(c + 1) * 512],
                                oT[:, di * S + qt * 128:di * S + (qt + 1) * 128],
                                wo_sb[:, di * D + c * 512:di * D + (c + 1) * 512],
                                start=(di == 0), stop=False)
                    for c in range(NC):
                        nc.tensor.matmul(
                            ps[:, c * 512:(c + 1) * 512], ones_r[:, 0:128],
                            cb_sb[:, c * 512:(c + 1) * 512],
                            start=False, stop=False)
                        nc.tensor.matmul(
                            ps[:, c * 512:(c + 1) * 512], ident,
                            xres[:, c * 512:(c + 1) * 512],
                            start=False, stop=True)
                    stats = ln_pool.tile([128, NC, 6], f32, name="stats", tag="st")
                    for c in range(NC):
                        nc.vector.bn_stats(stats[:, c, :],
                                           ps[:, c * 512:(c + 1) * 512])
                    mv = ln_pool.tile([128, 2], f32, name="mv", tag="mv")
                    nc.vector.bn_aggr(mv[:], stats[:])
                    rstd = ln_pool.tile([128, 1], f32, name="rstd", tag="rstd")
                    nc.scalar.activation(rstd[:], mv[:, 1:2], AF.Sqrt, bias=eps_t)
                    nc.vector.reciprocal(rstd[:], rstd[:])
                    # y = ((x - mu)*gamma) * rstd + beta via two fused
                    # scalar_tensor_tensor ops on DVE (2 insts instead of 3)
                    t2 = t_pool.tile([128, D], f32, name="t2", tag="t2")
                    nc.vector.scalar_tensor_tensor(
                        t2[:], ps[:], mv[:, 0:1], gamma_bc[:],
                        op0=mybir.AluOpType.subtract,
                        op1=mybir.AluOpType.mult)
                    t4 = t_pool.tile([128, D], f32, name="t4", tag="t4")
                    if qt < ST - 1:
                        # split the final fused op: rstd-scale on DVE, the
                        # plain beta-add on the idle GpSimd engine — lightens
                        # the DVE stream so the exposed tail chains shrink
                        t3 = t_pool.tile([128, D], f32, name="t3", tag="t3")
                        nc.vector.tensor_scalar_mul(t3[:], t2[:], rstd[:])
                        nc.gpsimd.tensor_add(t4[:], t3[:], beta_bc[:])
                    else:
                        # last tile: keep the short fused chain (its latency
                        # is fully exposed at kernel end)
                        nc.vector.scalar_tensor_tensor(
                            t4[:], t2[:], rstd[:], beta_bc[:],
                            op0=mybir.AluOpType.mult,
                            op1=mybir.AluOpType.add)
                    nc.sync.dma_start(y_ap[qt * 128:(qt + 1) * 128, :], t4[:])
            oT_pool.release()

    nc.compile()
    return nc


def _get_built():
    global _BUILT
    if _BUILT is None:
        _BUILT = _build()
    return _BUILT


def kernel(x1, x2, x3, mask, Wq, bq, Wk, bk, Wo, bo, gamma, beta):
    from concourse import bass_utils

    nc = _get_built()
    x1 = np.ascontiguousarray(np.asarray(x1, np.float32))
    x2 = np.ascontiguousarray(np.asarray(x2, np.float32))
    x3 = np.ascontiguousarray(np.asarray(x3, np.float32))
    mf = (np.asarray(mask) != 0).astype(np.float32)          # [B, 1, S]
    Wo_f = np.asarray(Wo, np.float32)
    bk_f = np.asarray(bk, np.float32).reshape(D)
    # V-bias (== bk per the reference) passes through Wo as a constant row;
    # fold it with bo (exact algebra, done once on host)
    cb = (bk_f @ Wo_f + np.asarray(bo, np.float32).reshape(D))
    shared = {
        "wq": np.ascontiguousarray(np.asarray(Wq, np.float32)),
        "wk": np.ascontiguousarray(np.asarray(Wk, np.float32)),
        "wo": np.ascontiguousarray(Wo_f),
        "bq": np.asarray(bq, np.float32).reshape(1, D),
        "bk": bk_f.reshape(1, D),
        "cb": np.ascontiguousarray(cb.reshape(1, D)),
        "gamma": np.asarray(gamma, np.float32).reshape(1, D),
        "beta": np.asarray(beta, np.float32).reshape(1, D),
    }
    in_maps = []
    for b in range(B):
        m = dict(shared)
        m["x1"] = x1[b]
        m["x2"] = x2[b]
        m["x3"] = x3[b]
        m["mf"] = np.ascontiguousarray(mf[b, 0, :].reshape(S, 1))
        in_maps.append(m)
    res = bass_utils.run_bass_kernel_spmd(nc, in_maps, core_ids=list(range(B)))
    return np.stack([res.results[b]["y"] for b in range(B)])

